# revision 14
# baseline (speedup 1.0000x reference)
"""Trainium2 Bass kernel for nn_Block_2010044694563 (dense transformer block).

B=4, S=2048, D=768, H=12 heads of 64. 8 NeuronCores, no collectives:
core c handles batch c//2, query-half c%2. Inputs rolled so the core's
1024 query rows come first; LN1 + K/V run over all 2048 local tokens.

Restructured pipeline vs the v1 kernel:
- Phase 1 streams per token-chunk: LN1 (batched stats), transposes, and
  V projection, with K/Q projections interleaved per 512-token group so
  the PE never idles and HAM stays warm (plus a warm-up transpose burst).
- Attention is software-pipelined: PV of chunk j-1 is issued under the
  scores of chunk j, and the softmax exp is split across two engines:
  head0 on ScalarE (ACT Exp) and head1 on DVE via a Schraudolph bf16
  bit-trick (round(x*(128/ln2) + C) as int16, bitcast to bf16; ~3.3% max
  rel err, washes out after softmax normalization).
- The post-attention tail (out-proj + LN2 + FFN) is fused per 512-query
  block and its work items are drip-fed into the next block's attention
  stream. LN2's rsqrt uses a DVE Newton bit-trick (no ACT table switch);
  FFN1 is evacuated to SBUF and GELU'd in one big clustered ACT call.
"""

import numpy as np
import ml_dtypes

B, S, D, H = 4, 2048, 768, 12
HS = D // H           # 64
P = 128
NT = S                # local tokens per core (whole batch)
NQ = S // 2           # query tokens per core
TCH = NT // P         # 16 token chunks
KC = D // P           # 6 feature chunks
EPS = 1e-5
NEG = -1e9
SCALE = float(D) ** -0.5
LOG2E = 1.4426950408889634
EXPA = 128.0 * LOG2E          # bf16 Schraudolph multiplier
EXPC = 16250.4                # calibrated shift (round-to-nearest conversion)
RSQC = 0x5F3759DF
BF16 = ml_dtypes.bfloat16

_PROGRAM_CACHE = {}
DEBUG_DUMPS = False


def _build_program():
    import concourse.bass as bass
    import concourse.mybir as mybir
    import concourse.tile as tile
    from concourse import bacc
    from concourse.masks import make_identity
    from contextlib import ExitStack

    f32 = mybir.dt.float32
    bf16 = mybir.dt.bfloat16
    i16 = mybir.dt.int16
    i32 = mybir.dt.int32
    AF = mybir.ActivationFunctionType
    OP = mybir.AluOpType

    nc = bacc.Bacc(None, target_bir_lowering=False)

    x_d = nc.dram_tensor("x_local", [NT, D], f32, kind="ExternalInput")
    mb_d = nc.dram_tensor("maskbias", [NT], f32, kind="ExternalInput")
    mbs_d = nc.dram_tensor("maskbias_s", [NT], f32, kind="ExternalInput")
    wq_d = nc.dram_tensor("wq", [D, D], bf16, kind="ExternalInput")
    wk_d = nc.dram_tensor("wk", [D, D], bf16, kind="ExternalInput")
    wv_d = nc.dram_tensor("wv", [D, D], bf16, kind="ExternalInput")
    wo_d = nc.dram_tensor("wo", [D, D], bf16, kind="ExternalInput")
    w1_d = nc.dram_tensor("w1", [D, D], bf16, kind="ExternalInput")
    w2_d = nc.dram_tensor("w2", [D, D], bf16, kind="ExternalInput")
    bq_d = nc.dram_tensor("bq", [D], f32, kind="ExternalInput")
    bk_d = nc.dram_tensor("bk", [D], f32, kind="ExternalInput")
    bo_d = nc.dram_tensor("bo2", [D], f32, kind="ExternalInput")
    b1_d = nc.dram_tensor("b1f", [D], f32, kind="ExternalInput")
    b2_d = nc.dram_tensor("b2f", [D], f32, kind="ExternalInput")
    out_d = nc.dram_tensor("out", [NQ, D], f32, kind="ExternalOutput")

    x_r = x_d[:].rearrange("(c p) d -> c p d", p=P)
    out_r = out_d[:].rearrange("(c p) d -> c p d", p=P)

    def split_dma(dst, src, n):
        # n-way split along the last (feature) axis -> n parallel DMA queues
        w = dst.shape[-1] // n
        pre_d = (slice(None),) * (len(dst.shape) - 1)
        pre_s = (slice(None),) * (len(src.shape) - 1)
        for q in range(n):
            sl = slice(q * w, (q + 1) * w)
            nc.sync.dma_start(out=dst[pre_d + (sl,)], in_=src[pre_s + (sl,)])

    with tile.TileContext(nc) as tc, ExitStack() as ctx:
        const = ctx.enter_context(tc.tile_pool(name="const", bufs=1))
        glob = ctx.enter_context(tc.tile_pool(name="glob", bufs=1))
        rot = ctx.enter_context(tc.tile_pool(name="rot", bufs=1))
        wpool = ctx.enter_context(tc.tile_pool(name="wpool", bufs=1))

        # ---- persistent tiles ----
        oT = glob.tile([P, KC, NQ], bf16)          # normalized attn out, feature-major
        x2 = glob.tile([P, NQ // P, D], f32)       # attn residual rows (x+attn@Wo+bo)

        apool = tc.alloc_tile_pool(name="apool", bufs=1)
        qT = apool.tile([P, KC, NQ], bf16)
        kT = apool.tile([P, KC, NT], bf16)
        vA = apool.tile([P, TCH, H, HS + 1], bf16)  # V per (tok chunk, head): [V | 1]

        # ---- input DMAs, earliest-needed first ----
        xts = []
        for t in range(4):
            xt = rot.tile([P, D], f32, tag="xt", bufs=4, name=f"xt{t}")
            split_dma(xt, x_r[t], 4)
            xts.append(xt)
        wv_sb = wpool.tile([P, KC, D], bf16, tag="w", bufs=3, name="wv_sb")
        split_dma(wv_sb, wv_d[:].rearrange("(c p) n -> p c n", p=P), 6)

        ident = const.tile([P, P], bf16)
        make_identity(nc, ident)
        mb_sb = const.tile([P, TCH], f32)
        nc.sync.dma_start(out=mb_sb, in_=mb_d[:].rearrange("(c p) -> p c", p=P))
        mbs_sb = const.tile([P, TCH], f32)
        nc.sync.dma_start(out=mbs_sb, in_=mbs_d[:].rearrange("(c p) -> p c", p=P))
        bq_sb = const.tile([P, KC], f32)
        nc.sync.dma_start(out=bq_sb, in_=bq_d[:].rearrange("(c p) -> p c", p=P))
        bk_sb = const.tile([P, KC], f32)
        nc.sync.dma_start(out=bk_sb, in_=bk_d[:].rearrange("(c p) -> p c", p=P))
        b1_sb = const.tile([P, KC], f32)
        nc.sync.dma_start(out=b1_sb, in_=b1_d[:].rearrange("(c p) -> p c", p=P))
        bo_b = const.tile([P, D], f32)
        _bo = bo_d[:]
        nc.gpsimd.dma_start(
            out=bo_b, in_=bass.AP(tensor=_bo.tensor, offset=_bo.offset, ap=[[0, P], _bo.ap[0]])
        )
        b2_b = const.tile([P, D], f32)
        _b2 = b2_d[:]
        nc.gpsimd.dma_start(
            out=b2_b, in_=bass.AP(tensor=_b2.tensor, offset=_b2.offset, ap=[[0, P], _b2.ap[0]])
        )

        wq_sb = wpool.tile([P, KC, D], bf16, tag="w", bufs=3, name="wq_sb")
        split_dma(wq_sb, wq_d[:].rearrange("(c p) n -> p c n", p=P), 6)
        wk_sb = wpool.tile([P, KC, D], bf16, tag="w", bufs=3, name="wk_sb")
        split_dma(wk_sb, wk_d[:].rearrange("(c p) n -> p c n", p=P), 6)
        for t in range(4, TCH):
            xt = rot.tile([P, D], f32, tag="xt", bufs=4, name=f"xt{t}")
            split_dma(xt, x_r[t], 4)
            xts.append(xt)

        nc.vector.memset(vA[:, :, :, HS : HS + 1], 1.0)

        hpool = tc.alloc_tile_pool(name="hpool", bufs=1)
        hT = hpool.tile([P, KC, NT], bf16)
        ps1 = tc.alloc_tile_pool(name="ps1", bufs=1, space="PSUM")

        # ---- HAM warm-up: dummy transposes keep PE active from t=0 ----
        for wi in range(20):
            wmm = ps1.tile([P, P], bf16, tag="warm", bufs=2, name=f"wmm{wi}")
            nc.tensor.transpose(wmm, ident, ident)

        # ================= Phase 1: LN1 + transposes + V/K/Q proj =============
        def ln_chunk(xt_c, dst, t):
            """LN a [P, D] chunk into dst (bf16) using batched per-pair stats."""
            scr = rot.tile([P, D], bf16, tag="xn", bufs=3, name=f"scr{t}")
            ssq = rot.tile([P, 1], f32, tag="ssq", bufs=4, name=f"ssq{t}")
            nc.scalar.activation(scr, xt_c, AF.Square, accum_out=ssq)
            msum = rot.tile([P, 1], f32, tag="msum", bufs=4, name=f"msum{t}")
            nc.vector.reduce_sum(out=msum, in_=xt_c, axis=mybir.AxisListType.X)
            nmu = rot.tile([P, 1], f32, tag="nmu", bufs=4, name=f"nmu{t}")
            nc.vector.tensor_scalar_mul(out=nmu, in0=msum, scalar1=-1.0 / D)
            ve = rot.tile([P, 1], f32, tag="ve", bufs=4, name=f"ve{t}")
            nc.vector.tensor_scalar(
                out=ve, in0=ssq, scalar1=1.0 / D, scalar2=EPS, op0=OP.mult, op1=OP.add
            )
            mu2 = rot.tile([P, 1], f32, tag="mu2", bufs=4, name=f"mu2{t}")
            nc.vector.tensor_tensor(mu2, nmu, nmu, OP.mult)
            nc.vector.tensor_tensor(ve, ve, mu2, OP.subtract)
            rstd = rot.tile([P, 1], f32, tag="rstd", bufs=4, name=f"rstd{t}")
            nc.vector.reciprocal_approx_fast(out=rstd, in_=ve)
            nc.scalar.activation(rstd, rstd, AF.Sqrt, scale=1.0)
            nmr = rot.tile([P, 1], f32, tag="nmr", bufs=4, name=f"nmr{t}")
            nc.vector.tensor_tensor(nmr, nmu, rstd, OP.mult)
            xn = rot.tile([P, D], bf16, tag="xn", bufs=3, name=f"xn{t}")
            nc.vector.tensor_scalar(
                out=xn, in0=xt_c, scalar1=rstd, scalar2=nmr, op0=OP.mult, op1=OP.add
            )
            pt = ps1.tile([P, KC, P], bf16, tag="pt", bufs=2, name=f"pt{t}")
            for f in range(KC):
                nc.tensor.transpose(pt[:, f], xn[:, f * P : (f + 1) * P], ident)
            nc.scalar.copy(out=dst, in_=pt)

        with nc.named_scope("ph1"):
            for t in range(TCH):
                ln_chunk(xts[t], hT[:, :, t * P : (t + 1) * P], t)
                # V projection for this chunk
                for n2 in range(2):
                    ps = ps1.tile([P, 512], f32, tag="pj", bufs=2, name=f"psv{t}_{n2}")
                    for kc in range(KC):
                        nc.tensor.matmul(
                            ps[:, :384],
                            lhsT=hT[:, kc, t * P : (t + 1) * P],
                            rhs=wv_sb[:, kc, n2 * 384 : (n2 + 1) * 384],
                            start=(kc == 0), stop=(kc == KC - 1),
                        )
                    nc.vector.tensor_copy(
                        out=vA[:, t, n2 * 6 : (n2 + 1) * 6, 0:HS],
                        in_=ps[:, :384].rearrange("p (h d) -> p h d", h=6),
                    )
                if t % 4 == 3:
                    g = t // 4
                    gs = slice(g * 512, (g + 1) * 512)
                    for hp in range(KC):
                        psk = ps1.tile([P, 512], f32, tag="pj", bufs=2, name=f"psk{g}_{hp}")
                        for kc in range(KC):
                            nc.tensor.matmul(
                                psk,
                                lhsT=wk_sb[:, kc, hp * P : (hp + 1) * P],
                                rhs=hT[:, kc, gs],
                                start=(kc == 0), stop=(kc == KC - 1),
                            )
                        if hp % 2 == 0:
                            nc.scalar.activation(
                                kT[:, hp, gs], psk, AF.Identity,
                                bias=bk_sb[:, hp : hp + 1], scale=1.0,
                            )
                        else:
                            nc.vector.tensor_scalar_add(
                                out=kT[:, hp, gs], in0=psk, scalar1=bk_sb[:, hp : hp + 1]
                            )
                    if g < 2:
                        for hp in range(KC):
                            psq = ps1.tile([P, 512], f32, tag="pj", bufs=2, name=f"psq{g}_{hp}")
                            for kc in range(KC):
                                nc.tensor.matmul(
                                    psq,
                                    lhsT=wq_sb[:, kc, hp * P : (hp + 1) * P],
                                    rhs=hT[:, kc, gs],
                                    start=(kc == 0), stop=(kc == KC - 1),
                                )
                            nc.vector.tensor_scalar_add(
                                out=qT[:, hp, gs], in0=psq, scalar1=bq_sb[:, hp : hp + 1]
                            )

        # weights for the tail, loaded into recycled wpool slots
        wo_sb = wpool.tile([P, KC, D], bf16, tag="w", bufs=3, name="wo_sb")
        split_dma(wo_sb, wo_d[:].rearrange("(c p) n -> p c n", p=P), 6)
        w1_sb = wpool.tile([P, KC, D], bf16, tag="w", bufs=3, name="w1_sb")
        split_dma(w1_sb, w1_d[:].rearrange("(c p) n -> p c n", p=P), 6)

        if DEBUG_DUMPS:
            hT_dump = nc.dram_tensor("d_hT", list(hT.shape), hT.dtype, kind="ExternalOutput")
            nc.sync.dma_start(out=hT_dump[:], in_=hT)

        ps1.release()
        hpool.release()

        lpool = tc.alloc_tile_pool(name="lpool", bufs=1)
        ps2 = tc.alloc_tile_pool(name="ps2", bufs=1, space="PSUM")
        dpool = tc.alloc_tile_pool(name="dpool", bufs=1, space="DRAM")

        w2_sb = wpool.tile([P, KC, D], bf16, tag="w", bufs=3, name="w2_sb")
        split_dma(w2_sb, w2_d[:].rearrange("(c p) n -> p c n", p=P), 6)

        # residual rows: DMA x into x2 and fold bo while attention runs
        for qm in range(NQ // P):
            split_dma(x2[:, qm], x_r[qm], 2)
        for qm in range(NQ // P):
            nc.vector.tensor_tensor(x2[:, qm], x2[:, qm], bo_b, OP.add)

        # ================= tail work items (per 512-query block) ==============
        def make_tail(qc):
            h2T = lpool.tile([P, KC, 512], bf16, tag="h2T", bufs=1, name=f"h2T{qc}")
            fT = lpool.tile([P, KC, 512], bf16, tag="fT", bufs=1, name=f"fT{qc}")
            gT = lpool.tile([P, KC, 512], bf16, tag="gT", bufs=1, name=f"gT{qc}")
            qms = list(range(qc * 4, qc * 4 + 4))
            items = []

            def proj(qm):
                def emit():
                    for n2 in range(2):
                        ns = slice(n2 * 384, (n2 + 1) * 384)
                        ps = ps2.tile([P, 512], f32, tag="tl", bufs=1, name=f"pso{qm}_{n2}")
                        for kc in range(KC):
                            nc.tensor.matmul(
                                ps[:, :384],
                                lhsT=oT[:, kc, qm * P : (qm + 1) * P],
                                rhs=wo_sb[:, kc, ns],
                                start=(kc == 0), stop=(kc == KC - 1),
                            )
                        nc.vector.tensor_tensor(
                            x2[:, qm, ns], ps[:, :384], x2[:, qm, ns], OP.add
                        )
                return emit
            for qm in qms:
                items.append(proj(qm))

            # LN2 batched stats for the 4 chunks of this block
            st = {}
            def ln2_stats():
                ssq = rot.tile([P, 4], f32, tag="ssq2", bufs=2, name=f"ssq2_{qc}")
                msum = rot.tile([P, 4], f32, tag="msum2", bufs=2, name=f"msum2_{qc}")
                for c, qm in enumerate(qms):
                    scr = rot.tile([P, D], bf16, tag="xn", bufs=3, name=f"scr2_{qm}")
                    nc.scalar.activation(
                        scr, x2[:, qm], AF.Square, accum_out=ssq[:, c : c + 1]
                    )
                nc.vector.reduce_sum(
                    out=msum, in_=x2[:, qc * 4 : qc * 4 + 4], axis=mybir.AxisListType.X
                )
                nmu = rot.tile([P, 4], f32, tag="nmu2", bufs=2, name=f"nmu2_{qc}")
                nc.vector.tensor_scalar_mul(out=nmu, in0=msum, scalar1=-1.0 / D)
                ve = rot.tile([P, 4], f32, tag="ve2", bufs=2, name=f"ve2_{qc}")
                nc.vector.tensor_scalar(
                    out=ve, in0=ssq, scalar1=1.0 / D, scalar2=EPS, op0=OP.mult, op1=OP.add
                )
                mu2 = rot.tile([P, 4], f32, tag="mu22", bufs=2, name=f"mu22_{qc}")
                nc.vector.tensor_tensor(mu2, nmu, nmu, OP.mult)
                nc.vector.tensor_tensor(ve, ve, mu2, OP.subtract)
                # rstd = rsqrt(ve) via DVE Newton bit-trick (no ACT table switch)
                sh = rot.tile([P, 4], i32, tag="sh2", bufs=2, name=f"sh2_{qc}")
                nc.vector.tensor_scalar(
                    out=sh, in0=ve.bitcast(i32), scalar1=1, scalar2=None,
                    op0=OP.arith_shift_right,
                )
                nc.vector.tensor_scalar(
                    out=sh, in0=sh, scalar1=RSQC, scalar2=-1, op0=OP.subtract, op1=OP.mult
                )
                y = sh.bitcast(f32)
                nh = rot.tile([P, 4], f32, tag="nh2", bufs=2, name=f"nh2_{qc}")
                nc.vector.tensor_scalar_mul(out=nh, in0=ve, scalar1=-0.5)
                p1 = rot.tile([P, 4], f32, tag="p12", bufs=2, name=f"p12_{qc}")
                for _ in range(2):
                    nc.vector.tensor_tensor(p1, y, y, OP.mult)
                    nc.vector.tensor_tensor(p1, p1, nh, OP.mult)
                    nc.vector.tensor_scalar_add(out=p1, in0=p1, scalar1=1.5)
                    nc.vector.tensor_tensor(y, y, p1, OP.mult)
                nmr = rot.tile([P, 4], f32, tag="nmr2", bufs=2, name=f"nmr2_{qc}")
                nc.vector.tensor_tensor(nmr, nmu, y, OP.mult)
                st["rstd"], st["nmr"] = y, nmr
            items.append(ln2_stats)

            def ln2_xn(c, qm):
                def emit():
                    xn = rot.tile([P, D], bf16, tag="xn", bufs=3, name=f"xn2_{qm}")
                    nc.vector.tensor_scalar(
                        out=xn, in0=x2[:, qm],
                        scalar1=st["rstd"][:, c : c + 1], scalar2=st["nmr"][:, c : c + 1],
                        op0=OP.mult, op1=OP.add,
                    )
                    nc.vector.tensor_tensor(x2[:, qm], x2[:, qm], b2_b, OP.add)
                    pt = ps2.tile([P, KC, P], bf16, tag="tl", bufs=1, name=f"pt2_{qm}")
                    for f in range(KC):
                        nc.tensor.transpose(pt[:, f], xn[:, f * P : (f + 1) * P], ident)
                    nc.vector.tensor_copy(out=h2T[:, :, c * P : (c + 1) * P], in_=pt)
                return emit
            for c, qm in enumerate(qms):
                items.append(ln2_xn(c, qm))

            def ffn1(m):
                def emit():
                    ps = ps2.tile([P, 512], f32, tag="tl", bufs=1, name=f"psf{qc}_{m}")
                    for kc in range(KC):
                        nc.tensor.matmul(
                            ps,
                            lhsT=w1_sb[:, kc, m * P : (m + 1) * P],
                            rhs=h2T[:, kc, :],
                            start=(kc == 0), stop=(kc == KC - 1),
                        )
                    nc.vector.tensor_scalar_add(
                        out=fT[:, m], in0=ps, scalar1=b1_sb[:, m : m + 1]
                    )
                return emit
            for m in range(KC):
                items.append(ffn1(m))

            def gelu_cluster():
                nc.scalar.activation(gT[:], fT[:], AF.Gelu, scale=1.0)
            items.append(gelu_cluster)

            def ffn2(qm):
                def emit():
                    lq = qm - qc * 4
                    osb = rot.tile([P, D], f32, tag="osb", bufs=2, name=f"osb{qm}")
                    for n2 in range(2):
                        ns = slice(n2 * 384, (n2 + 1) * 384)
                        ps = ps2.tile([P, 512], f32, tag="tl", bufs=1, name=f"psg{qm}_{n2}")
                        for kc in range(KC):
                            nc.tensor.matmul(
                                ps[:, :384],
                                lhsT=gT[:, kc, lq * P : (lq + 1) * P],
                                rhs=w2_sb[:, kc, ns],
                                start=(kc == 0), stop=(kc == KC - 1),
                            )
                        nc.vector.tensor_tensor(osb[:, ns], ps[:, :384], x2[:, qm, ns], OP.add)
                    split_dma(out_r[qm], osb, 4)
                return emit
            for qm in qms:
                items.append(ffn2(qm))
            return items

        # ================= attention (qc outer, software-pipelined) ===========
        pending_pv = None
        pending_norm = None
        tail_queue = []

        def make_pv(hp, qc, j, pv0, pv1, ex0, ex1):
            def emit():
                nc.tensor.matmul(
                    pv0, lhsT=vA[:, j, 2 * hp, :], rhs=ex0,
                    start=(j == 0), stop=(j == TCH - 1),
                )
                nc.tensor.matmul(
                    pv1, lhsT=vA[:, j, 2 * hp + 1, :], rhs=ex1,
                    start=(j == 0), stop=(j == TCH - 1),
                )
            return emit

        def make_norm(hp, qc, pv0, pv1):
            def emit():
                qs = slice(qc * 512, (qc + 1) * 512)
                pvr = rot.tile([1, 2, 512], f32, tag="pvr", bufs=2, name=f"pvr{hp}_{qc}")
                nc.vector.tensor_copy(out=pvr[:, 0, :], in_=pv0[HS : HS + 1, :])
                nc.vector.tensor_copy(out=pvr[:, 1, :], in_=pv1[HS : HS + 1, :])
                rsb = rot.tile([1, 2, 512], f32, tag="rsb", bufs=2, name=f"rsb{hp}_{qc}")
                nc.vector.reciprocal_approx_fast(out=rsb[:, 0, :], in_=pvr[:, 0, :])
                nc.vector.reciprocal_approx_fast(out=rsb[:, 1, :], in_=pvr[:, 1, :])
                rd = dpool.tile([1, 2, 512], f32, tag="rd", bufs=2, name=f"rd{hp}_{qc}")
                nc.sync.dma_start(out=rd, in_=rsb)
                rrs = rot.tile([HS, 2, 512], f32, tag="rrs", bufs=2, name=f"rrs{hp}_{qc}")
                nc.gpsimd.dma_start(
                    out=rrs,
                    in_=bass.AP(
                        tensor=rd.tensor, offset=rd.offset,
                        ap=[[0, HS]] + [list(a) for a in rd.ap[1:]],
                    ),
                )
                nc.vector.tensor_tensor(oT[0:HS, hp, qs], pv0[0:HS, :], rrs[:, 0, :], OP.mult)
                nc.vector.tensor_tensor(oT[HS:P, hp, qs], pv1[0:HS, :], rrs[:, 1, :], OP.mult)
            return emit

        for qc in range(2):
            with nc.named_scope(f"attn{qc}"):
                qs = slice(qc * 512, (qc + 1) * 512)
                for hp in range(KC):
                    pv0 = ps2.tile([HS + 1, 512], f32, tag="pv", bufs=4, name=f"pv0_{hp}_{qc}")
                    pv1 = ps2.tile([HS + 1, 512], f32, tag="pv", bufs=4, name=f"pv1_{hp}_{qc}")
                    for j in range(TCH):
                        js = slice(j * P, (j + 1) * P)
                        sc0 = ps2.tile([P, 512], f32, tag="sc", bufs=3, name=f"sc0_{hp}_{qc}_{j}")
                        sc1 = ps2.tile([P, 512], f32, tag="sc", bufs=3, name=f"sc1_{hp}_{qc}_{j}")
                        nc.tensor.matmul(
                            sc0, lhsT=kT[0:HS, hp, js], rhs=qT[0:HS, hp, qs],
                            start=True, stop=True,
                        )
                        nc.tensor.matmul(
                            sc1, lhsT=kT[HS:P, hp, js], rhs=qT[HS:P, hp, qs],
                            start=True, stop=True,
                        )
                        ex0 = rot.tile([P, 512], bf16, tag="ex", bufs=4, name=f"ex0_{hp}_{qc}_{j}")
                        nc.scalar.activation(
                            ex0, sc0, AF.Exp, bias=mb_sb[:, j : j + 1], scale=SCALE
                        )
                        ex1i = rot.tile([P, 512], i16, tag="ex", bufs=4, name=f"ex1_{hp}_{qc}_{j}")
                        nc.vector.tensor_scalar(
                            out=ex1i, in0=sc1, scalar1=SCALE * EXPA,
                            scalar2=mbs_sb[:, j : j + 1], op0=OP.mult, op1=OP.add,
                        )
                        if pending_pv is not None:
                            pending_pv()
                        pending_pv = make_pv(hp, qc, j, pv0, pv1, ex0, ex1i.bitcast(bf16))
                        # tail(0) items only once qc1/hp>=1: the last qc0 norm
                        # (oT writer) is emitted at the end of qc1/hp0
                        if qc == 1 and hp >= 1 and j in (2, 5, 8, 11, 14) and tail_queue:
                            tail_queue.pop(0)()
                    if pending_norm is not None:
                        pending_norm()
                    pending_norm = make_norm(hp, qc, pv0, pv1)
            if qc == 0:
                tail_queue.extend(make_tail(0))

        pending_pv()
        pending_norm()

        if DEBUG_DUMPS:
            dbg = {
                "d_qT": qT, "d_kT": kT, "d_vA": vA, "d_oT": oT,
            }
            for nm, t in dbg.items():
                td = nc.dram_tensor(nm, list(t.shape), t.dtype, kind="ExternalOutput")
                nc.sync.dma_start(out=td[:], in_=t)

        with nc.named_scope("tail"):
            for item in tail_queue:
                item()
            for item in make_tail(1):
                item()

        if DEBUG_DUMPS:
            x2_dump = nc.dram_tensor("d_x2", list(x2.shape), x2.dtype, kind="ExternalOutput")
            nc.sync.dma_start(out=x2_dump[:], in_=x2)

        lpool.release()
        ps2.release()
        dpool.release()
        apool.release()

    nc.finalize()
    return nc


def _prepare_in_maps(inputs):
    x = np.ascontiguousarray(np.asarray(inputs["x"], dtype=np.float32))
    mask = np.asarray(inputs["attention_mask"])
    ln1_g = np.asarray(inputs["ln1_g"], dtype=np.float64)
    ln1_b = np.asarray(inputs["ln1_b"], dtype=np.float64)
    ln2_g = np.asarray(inputs["ln2_g"], dtype=np.float64)
    ln2_b = np.asarray(inputs["ln2_b"], dtype=np.float64)
    Wq = np.asarray(inputs["Wq"], dtype=np.float64)
    Wk = np.asarray(inputs["Wk"], dtype=np.float64)
    Wv = np.asarray(inputs["Wv"], dtype=np.float64)
    Wo = np.asarray(inputs["Wo"], dtype=np.float64)
    W1 = np.asarray(inputs["W1"], dtype=np.float64)
    W2 = np.asarray(inputs["W2"], dtype=np.float64)
    bo = np.asarray(inputs["bo"], dtype=np.float64)
    b1 = np.asarray(inputs["b1"], dtype=np.float64)
    b2 = np.asarray(inputs["b2"], dtype=np.float64)

    # fold LN gains/biases into the projection weights
    wq_f = (ln1_g[:, None] * Wq).astype(BF16)
    wk_f = (ln1_g[:, None] * Wk).astype(BF16)
    wv_f = (ln1_g[:, None] * Wv).astype(BF16)
    bq = (ln1_b @ Wq).astype(np.float32)
    bk = (ln1_b @ Wk).astype(np.float32)
    bv = ln1_b @ Wv
    wo_f = Wo.astype(BF16)
    bo2 = (bo + bv @ Wo).astype(np.float32)  # V-bias adds uniformly post-softmax
    w1_f = (ln2_g[:, None] * W1).astype(BF16)
    b1f = (b1 + ln2_b @ W1).astype(np.float32)
    w2_f = W2.astype(BF16)
    b2f = b2.astype(np.float32)

    maskbias = np.where(mask == 0, np.float32(NEG), np.float32(0.0)).astype(np.float32)
    # Schraudolph-domain mask bias: logit*EXPA + EXPC, or a large negative
    # that saturates the int16 convert into a ~zero bf16 for masked keys
    maskbias_s = np.where(mask == 0, np.float32(-20000.0), np.float32(EXPC)).astype(
        np.float32
    )

    in_maps = []
    for c in range(8):
        b, half = divmod(c, 2)
        xb = np.roll(x[b], -half * NQ, axis=0)
        mbb = np.roll(maskbias[b], -half * NQ, axis=0)
        mbsb = np.roll(maskbias_s[b], -half * NQ, axis=0)
        in_maps.append(
            {
                "x_local": np.ascontiguousarray(xb),
                "maskbias": np.ascontiguousarray(mbb),
                "maskbias_s": np.ascontiguousarray(mbsb),
                "wq": wq_f, "wk": wk_f, "wv": wv_f, "wo": wo_f,
                "w1": w1_f, "w2": w2_f,
                "bq": bq, "bk": bk, "bo2": bo2, "b1f": b1f, "b2f": b2f,
            }
        )
    return in_maps


def run_on_cores(inputs, **spmd_kwargs):
    """Build (cached), run on cores 0-7, return (full_output, BassKernelResults)."""
    from concourse.bass_utils import run_bass_kernel_spmd

    if "nc" not in _PROGRAM_CACHE:
        _PROGRAM_CACHE["nc"] = _build_program()
    nc = _PROGRAM_CACHE["nc"]
    in_maps = _prepare_in_maps(inputs)
    res = run_bass_kernel_spmd(nc, in_maps, core_ids=list(range(8)), **spmd_kwargs)
    out = np.empty((B, S, D), dtype=np.float32)
    for c in range(8):
        b, half = divmod(c, 2)
        out[b, half * NQ : (half + 1) * NQ] = res.results[c]["out"]
    return out, res


def kernel(**inputs):
    out, _ = run_on_cores(inputs)
    return out
